# revision 25
# baseline (speedup 1.0000x reference)
"""Trainium2 Bass kernel for nn_ActorCriticTransformer.

Data-parallel over 8 NeuronCores: each core runs the full network on a
4096-row slice of the batch.  Weights are replicated.

Layout strategy per core:
  - Residual stream h kept FEATURE-major [128 d, 512 b] per macro-chunk.
  - x is loaded batch-major and transposed on the PE (4 x [128,~128] chunks).
  - LayerNorm stats via PE ones-matmul (M=1), rsqrt via ACT ln/exp
    (single activation table set for the whole kernel), broadcast back
    across partitions via K=1 PE matmuls.
  - q,k,v produced batch-major [128 b, 384] (lhsT = y feature-major tile).
  - Outer-product attention computed batch-major with stride-0 broadcast
    APs: P[b,(h,i,j)] = q[b,(h,i)]*k[b,(h,j)] (DVE), E = exp(s*P) (ACT),
    segmented tensor_reduce over j for S and O (DVE), softmax division via
    exp(-ln(S)) (ACT).
  - LN gains/biases folded into the adjacent weights on the host.
  - Policy head folds pol_w @ word_matrix.T on-device once, then
    h.T @ polw_eff streams straight from PSUM to HBM via DMA.
"""

import os
import sys

for _p in ("/opt/trn_rl_repo", "/root/.axon_site/_ro/trn_rl_repo"):
    if os.path.isdir(_p) and _p not in sys.path:
        sys.path.insert(0, _p)

import numpy as np

B = 32768
DIN = 417
D = 128
H = 4
DH = 32
NB = 2
V = 2315
POL = 130
FF = 512
EPS = 1e-5
SCALE = DH ** -0.5
NCORES = 8
BC = B // NCORES          # rows per core
MCB = 512                 # macro-chunk batch size
DIN_CHUNKS = [(0, 128), (128, 128), (256, 128), (384, 33)]
V_CHUNKS = [(0, 512), (512, 512), (1024, 512), (1536, 512), (2048, V - 2048)]

_PROGRAM_CACHE = {}


def _build_program(bc, skip_bqkv, skip_bo, skip_b2, skip_polb, skip_b1):
    import concourse.bacc as bacc
    import concourse.tile as tile
    from concourse import mybir
    import concourse.bass as bass
    from concourse.masks import make_identity

    f32 = mybir.dt.float32
    AF = mybir.ActivationFunctionType
    ALU = mybir.AluOpType
    AX = mybir.AxisListType

    nc = bacc.Bacc(None, target_bir_lowering=False, debug=False)

    x_d = nc.dram_tensor("x", [bc, DIN], f32, kind="ExternalInput")
    encw_d = nc.dram_tensor("encw", [DIN, D], f32, kind="ExternalInput")
    encb_d = nc.dram_tensor("encb", [D, 1], f32, kind="ExternalInput")
    wqkv_d = nc.dram_tensor("wqkv", [NB, D, 3 * D], f32, kind="ExternalInput")
    bqkv_d = nc.dram_tensor("bqkv", [NB, 1, 3 * D], f32, kind="ExternalInput")
    wo_d = nc.dram_tensor("wo", [NB, D, D], f32, kind="ExternalInput")
    bo_d = nc.dram_tensor("bo", [NB, 1, D], f32, kind="ExternalInput")
    w1_d = nc.dram_tensor("w1", [NB, D, FF], f32, kind="ExternalInput")
    b1_d = nc.dram_tensor("b1", [NB, FF, 1], f32, kind="ExternalInput")
    w2_d = nc.dram_tensor("w2", [NB, FF, D], f32, kind="ExternalInput")
    b2_d = nc.dram_tensor("b2", [NB, 1, D], f32, kind="ExternalInput")
    valw_d = nc.dram_tensor("valw", [D, 1], f32, kind="ExternalInput")
    valb_d = nc.dram_tensor("valb", [1, 1], f32, kind="ExternalInput")
    polwT_d = nc.dram_tensor("polwT", [POL, D], f32, kind="ExternalInput")
    polb_d = nc.dram_tensor("polb", [POL, 1], f32, kind="ExternalInput")
    wmT_d = nc.dram_tensor("wmT", [POL, V], f32, kind="ExternalInput")

    policy_d = nc.dram_tensor("policy", [bc, V], f32, kind="ExternalOutput")
    value_d = nc.dram_tensor("value", [bc, 1], f32, kind="ExternalOutput")

    n_mc = bc // MCB

    from contextlib import ExitStack
    with tile.TileContext(nc) as tc:
        with (
            tc.tile_pool(name="singles", bufs=1) as singles,
            tc.tile_pool(name="psA", bufs=2, space="PSUM") as psA,
            tc.tile_pool(name="psB", bufs=3, space="PSUM") as psB,
            tc.tile_pool(name="psC", bufs=2, space="PSUM") as psC,
            tc.tile_pool(name="psD", bufs=1, space="PSUM") as psD,
        ):
            # ---------------- one-time: weights to SBUF ----------------
            # (deprioritized so the first macro-chunk's x-DMA/encoder isn't
            # stuck behind ~2MB of weight DMA + the policy fold)
            _depri = tc.high_priority(offset=-300)
            _depri.__enter__()
            encw_sb = singles.tile([128, 4, D], f32)
            for c, (c0, cw) in enumerate(DIN_CHUNKS):
                nc.sync.dma_start(encw_sb[:cw, c, :], encw_d[c0:c0 + cw, :])
            encb_sb = singles.tile([D, 1], f32)
            nc.sync.dma_start(encb_sb[:], encb_d[:])
            wqkv_sb = singles.tile([D, NB, 3 * D], f32)
            wo_sb = singles.tile([D, NB, D], f32)
            w1_sb = singles.tile([D, NB, FF], f32)
            w2_sb = singles.tile([D, NB, 4, D], f32)
            b1_sb = singles.tile([D, NB, 4, 1], f32)
            for i in range(NB):
                nc.sync.dma_start(wqkv_sb[:, i, :], wqkv_d[i])
                nc.sync.dma_start(wo_sb[:, i, :], wo_d[i])
                nc.sync.dma_start(w1_sb[:, i, :], w1_d[i])
                for c in range(4):
                    nc.sync.dma_start(w2_sb[:, i, c, :], w2_d[i, c * 128:(c + 1) * 128, :])
                    nc.sync.dma_start(b1_sb[:, i, c, :], b1_d[i, c * 128:(c + 1) * 128, :])
            if not skip_bqkv:
                bqkv_sb = singles.tile([1, NB, 3 * D], f32)
                for i in range(NB):
                    nc.sync.dma_start(bqkv_sb[:, i, :], bqkv_d[i])
            if not skip_bo:
                bo_sb = singles.tile([1, NB, D], f32)
                for i in range(NB):
                    nc.sync.dma_start(bo_sb[:, i, :], bo_d[i])
            if not skip_b2:
                b2_sb = singles.tile([1, NB, D], f32)
                for i in range(NB):
                    nc.sync.dma_start(b2_sb[:, i, :], b2_d[i])
            valw_sb = singles.tile([D, 1], f32)
            nc.sync.dma_start(valw_sb[:], valw_d[:])
            valb_sb = singles.tile([1, 1], f32)
            nc.sync.dma_start(valb_sb[:], valb_d[:])

            ident = singles.tile([128, 128], f32)
            with tc.high_priority():
                make_identity(nc, ident[:])
            ones_row = singles.tile([1, 128], f32)
            nc.vector.memset(ones_row[:], 1.0)
            ones512 = singles.tile([1, MCB], f32)
            nc.vector.memset(ones512[:], 1.0)
            inv_col = singles.tile([128, 1], f32)
            nc.vector.memset(inv_col[:], 1.0 / D)
            zero_col = singles.tile([128, 1], f32)
            nc.vector.memset(zero_col[:], 0.0)
            zero1 = singles.tile([1, 1], f32)
            nc.vector.memset(zero1[:], 0.0)
            eps1 = singles.tile([1, 1], f32)
            nc.vector.memset(eps1[:], EPS)

            # ---------------- one-time: policy weight fold ----------------
            # The word-matrix tiles are only needed here; a scoped pool gives
            # the ~20KB/partition back to the main loop afterwards.
            polw_eff = singles.tile([128, V], f32)
            if not skip_polb:
                polb_eff = singles.tile([1, V], f32)
            with tc.tile_pool(name="foldp", bufs=1) as foldp:
                polwT_sb = foldp.tile([128, 2, D], f32)
                nc.sync.dma_start(polwT_sb[:, 0, :], polwT_d[0:128, :])
                nc.sync.dma_start(polwT_sb[:POL - 128, 1, :], polwT_d[128:POL, :])
                wmT_sb = foldp.tile([128, 2, V], f32)
                nc.sync.dma_start(wmT_sb[:, 0, :], wmT_d[0:128, :])
                nc.sync.dma_start(wmT_sb[:POL - 128, 1, :], wmT_d[128:POL, :])
                if not skip_polb:
                    polb_sb = foldp.tile([128, 2, 1], f32)
                    nc.sync.dma_start(polb_sb[:, 0, :], polb_d[0:128, :])
                    nc.sync.dma_start(polb_sb[:POL - 128, 1, :], polb_d[128:POL, :])
                for n0, nw in V_CHUNKS:
                    pw_ps = psA.tile([128, 512], f32, tag="psa")
                    nc.tensor.matmul(pw_ps[:, :nw], polwT_sb[:, 0, :],
                                     wmT_sb[:, 0, n0:n0 + nw], start=True, stop=False)
                    nc.tensor.matmul(pw_ps[:, :nw], polwT_sb[:POL - 128, 1, :],
                                     wmT_sb[:POL - 128, 1, n0:n0 + nw], start=False, stop=True)
                    nc.scalar.copy(polw_eff[:, n0:n0 + nw], pw_ps[:, :nw])
                    if not skip_polb:
                        pb_ps = psC.tile([1, 512], f32, tag="psc")
                        nc.tensor.matmul(pb_ps[:, :nw], polb_sb[:, 0, :],
                                         wmT_sb[:, 0, n0:n0 + nw], start=True, stop=False)
                        nc.tensor.matmul(pb_ps[:, :nw], polb_sb[:POL - 128, 1, :],
                                         wmT_sb[:POL - 128, 1, n0:n0 + nw], start=False, stop=True)
                        nc.vector.tensor_copy(polb_eff[:, n0:n0 + nw], pb_ps[:, :nw])

            _depri.__exit__(None, None, None)

            _main_pools = ExitStack()
            xp = _main_pools.enter_context(tc.tile_pool(name="xp", bufs=2))
            xtp = _main_pools.enter_context(tc.tile_pool(name="xtp", bufs=1))
            hp = _main_pools.enter_context(tc.tile_pool(name="hp", bufs=4))
            lnp = _main_pools.enter_context(tc.tile_pool(name="lnp", bufs=3))
            lns = _main_pools.enter_context(tc.tile_pool(name="lns", bufs=3))
            yp = _main_pools.enter_context(tc.tile_pool(name="yp", bufs=3))
            qp = _main_pools.enter_context(tc.tile_pool(name="qp", bufs=4))
            bigp = _main_pools.enter_context(tc.tile_pool(name="bigp", bufs=2))
            bige = _main_pools.enter_context(tc.tile_pool(name="bige", bufs=2))
            smp = _main_pools.enter_context(tc.tile_pool(name="smp", bufs=4))
            ofp = _main_pools.enter_context(tc.tile_pool(name="ofp", bufs=3))
            ffp = _main_pools.enter_context(tc.tile_pool(name="ffp", bufs=2))

            # ---------------- layernorm (feature-major, width-parametric) -------
            def layernorm(h_ap, W):
                hsq = lnp.tile([D, W], f32, tag="hsq")
                nc.vector.tensor_mul(hsq[:], h_ap, h_ap)
                mu_ps = psC.tile([1, W], f32, tag="psc")
                m2_ps = psC.tile([1, W], f32, tag="psc")
                nc.tensor.matmul(mu_ps[:], inv_col[:], h_ap, start=True, stop=True)
                nc.tensor.matmul(m2_ps[:], inv_col[:], hsq[:], start=True, stop=True)
                mu = lns.tile([1, W], f32, tag="mu")
                nc.scalar.copy(mu[:], mu_ps[:])
                # var = m2 - mu^2 ; rstd = exp(-0.5*ln(var+eps))
                musq = lns.tile([1, W], f32, tag="musq")
                nc.scalar.activation(musq[:], mu_ps[:], AF.Square, bias=zero1[:])
                var = lns.tile([1, W], f32, tag="var")
                nc.vector.tensor_sub(var[:], m2_ps[:], musq[:])
                lnv = lns.tile([1, W], f32, tag="lnv")
                nc.scalar.activation(lnv[:], var[:], AF.Ln, bias=eps1[:])
                rstd = lns.tile([1, W], f32, tag="rstd")
                nc.scalar.activation(rstd[:], lnv[:], AF.Exp, bias=zero1[:], scale=-0.5)
                mu_bc = psB.tile([128, W], f32, tag="psb")
                rs_bc = psB.tile([128, W], f32, tag="psb")
                nc.tensor.matmul(mu_bc[:], ones_row[:], mu[:], start=True, stop=True)
                nc.tensor.matmul(rs_bc[:], ones_row[:], rstd[:], start=True, stop=True)
                t = lnp.tile([D, W], f32, tag="t")
                nc.vector.tensor_sub(t[:], h_ap, mu_bc[:])
                y = yp.tile([D, W], f32, tag="y")
                nc.vector.tensor_mul(y[:], t[:], rs_bc[:])
                return y

            # ---------------- attention stages (per 128-sample subtile) --------
            def stage_a(y, loc, blk):
                qkv_ps = psB.tile([128, 3 * D], f32, tag="psb")
                nc.tensor.matmul(qkv_ps[:], y[:, loc * 128:(loc + 1) * 128],
                                 wqkv_sb[:, blk, :], start=True, stop=skip_bqkv)
                if not skip_bqkv:
                    nc.tensor.matmul(qkv_ps[:], ones_row[:], bqkv_sb[:, blk, :],
                                     start=False, stop=True)
                qkv = qp.tile([128, 3 * D], f32, tag="qkv")
                nc.scalar.copy(qkv[:], qkv_ps[:])
                q_ap = qkv[:, 0:D]
                k_ap = qkv[:, D:2 * D]
                q_bc = bass.AP(tensor=q_ap.tensor, offset=q_ap.offset,
                               ap=[q_ap.ap[0], [DH, H], [1, DH], [0, DH]])
                k_bc = bass.AP(tensor=k_ap.tensor, offset=k_ap.offset,
                               ap=[k_ap.ap[0], [DH, H], [0, DH], [1, DH]])
                P = bigp.tile([128, H * DH * DH], f32, tag="P")
                nc.vector.tensor_mul(
                    P[:].rearrange("p (h i j) -> p h i j", h=H, i=DH), q_bc, k_bc)
                E = bige.tile([128, H * DH * DH], f32, tag="E")
                nc.scalar.activation(E[:], P[:], AF.Exp, bias=zero_col[:], scale=SCALE)
                return {"qkv": qkv, "P": P, "E": E}

            def stage_b(st):
                S = smp.tile([128, D], f32, tag="S")
                nc.vector.tensor_reduce(
                    S[:], st["E"][:].rearrange("p (a j) -> p a j", j=DH),
                    axis=AX.X, op=ALU.add)
                v_ap = st["qkv"][:, 2 * D:3 * D]
                v_bc = bass.AP(tensor=v_ap.tensor, offset=v_ap.offset,
                               ap=[v_ap.ap[0], [DH, H], [0, DH], [1, DH]])
                # EV into P's slot (dead after exp); GPSIMD is otherwise idle.
                nc.gpsimd.tensor_mul(
                    st["P"][:].rearrange("p (h i j) -> p h i j", h=H, i=DH),
                    st["E"][:].rearrange("p (h i j) -> p h i j", h=H, i=DH), v_bc)
                lnS = smp.tile([128, D], f32, tag="lnS")
                nc.scalar.activation(lnS[:], S[:], AF.Ln, bias=zero_col[:])
                rS = smp.tile([128, D], f32, tag="rS")
                nc.scalar.activation(rS[:], lnS[:], AF.Exp, bias=zero_col[:], scale=-1.0)
                st["rS"] = rS

            def stage_c(st, ofm, loc):
                O = smp.tile([128, D], f32, tag="O")
                nc.vector.tensor_reduce(
                    O[:], st["P"][:].rearrange("p (a j) -> p a j", j=DH),
                    axis=AX.X, op=ALU.add)
                On = smp.tile([128, D], f32, tag="On")
                nc.vector.tensor_mul(On[:], O[:], st["rS"][:])
                ot_ps = psC.tile([128, D], f32, tag="psc")
                nc.tensor.transpose(ot_ps[:], On[:], ident[:])
                nc.scalar.copy(ofm[:, loc * 128:(loc + 1) * 128], ot_ps[:])

            # ---------------- one batch stream (W columns of a macro-chunk) ----
            # Two 256-wide streams run phase-shifted so that one stream's
            # attention (DVE-heavy) fills the other's LayerNorm/FF bubbles.
            SW = 256      # stream width
            NSUB = SW // 128

            def stream(h0_ap, b0, c0):
                h_ap = h0_ap
                for blk in range(NB):
                    y = layernorm(h_ap, SW)
                    yield
                    sts = [stage_a(y, ls, blk) for ls in range(NSUB)]
                    yield
                    for ls in range(NSUB):
                        stage_b(sts[ls])
                    yield
                    ofm = ofp.tile([D, SW], f32, tag="ofm")
                    for ls in range(NSUB):
                        stage_c(sts[ls], ofm, ls)
                    wo_ps = psA.tile([128, SW], f32, tag="psa")
                    nc.tensor.matmul(wo_ps[:], wo_sb[:, blk, :], ofm[:],
                                     start=True, stop=skip_bo)
                    if not skip_bo:
                        nc.tensor.matmul(wo_ps[:], bo_sb[:, blk, :], ones512[:, :SW],
                                         start=False, stop=True)
                    h2 = hp.tile([D, SW], f32, tag="h")
                    nc.vector.tensor_add(h2[:], h_ap, wo_ps[:])
                    h_ap = h2[:]
                    yield
                    y2 = layernorm(h_ap, SW)
                    yield
                    ffs = ffp.tile([128, 4, SW], f32, tag="ffs")
                    w2_ps = psA.tile([128, SW], f32, tag="psa")
                    for c in range(4):
                        ff_ps = psB.tile([128, SW], f32, tag="psb")
                        nc.tensor.matmul(ff_ps[:], w1_sb[:, blk, c * 128:(c + 1) * 128],
                                         y2[:], start=True, stop=True)
                        if skip_b1:
                            nc.vector.tensor_relu(ffs[:, c, :], ff_ps[:])
                        else:
                            nc.scalar.activation(ffs[:, c, :], ff_ps[:], AF.Relu,
                                                 bias=b1_sb[:, blk, c, :])
                        nc.tensor.matmul(w2_ps[:], w2_sb[:, blk, c, :], ffs[:, c, :],
                                         start=(c == 0), stop=(c == 3 and skip_b2))
                    if not skip_b2:
                        nc.tensor.matmul(w2_ps[:], b2_sb[:, blk, :], ones512[:, :SW],
                                         start=False, stop=True)
                    h3 = hp.tile([D, SW], f32, tag="h")
                    nc.vector.tensor_add(h3[:], h_ap, w2_ps[:])
                    h_ap = h3[:]
                    yield
                # heads
                val_ps = psC.tile([1, SW], f32, tag="psc")
                nc.tensor.matmul(val_ps[:], valw_sb[:], h_ap, start=True, stop=True)
                val_sb = lns.tile([1, SW], f32, tag="val")
                nc.scalar.activation(val_sb[:], val_ps[:], AF.Identity, bias=valb_sb[:])
                nc.sync.dma_start(value_d[b0 + c0:b0 + c0 + SW, :], val_sb[:])
                with tc.high_priority(offset=-150):
                    for ls in range(NSUB):
                        for n0, nw in V_CHUNKS:
                            pol_ps = psD.tile([128, 512], f32, tag="psd")
                            nc.tensor.matmul(pol_ps[:, :nw],
                                             h_ap[:, ls * 128:(ls + 1) * 128],
                                             polw_eff[:, n0:n0 + nw], start=True,
                                             stop=skip_polb)
                            if not skip_polb:
                                nc.tensor.matmul(pol_ps[:, :nw], ones_row[:],
                                                 polb_eff[:, n0:n0 + nw], start=False,
                                                 stop=True)
                            pol_sb = ofp.tile([128, 512], f32, tag="pol")
                            nc.scalar.copy(pol_sb[:, :nw], pol_ps[:, :nw])
                            nc.sync.dma_start(
                                policy_d[b0 + c0 + ls * 128:b0 + c0 + (ls + 1) * 128,
                                         n0:n0 + nw],
                                pol_sb[:, :nw])
                yield

            # ---------------- main loop ----------------
            for mc in range(n_mc):
                b0 = mc * MCB
                # Emit the load/transpose/encode front-end "earlier" so it fills
                # the previous macro-chunk's FF/LN engine bubbles.
                with tc.high_priority(offset=(250 if mc > 0 else None)):
                    x_sb = xp.tile([128, 4, DIN], f32, tag="x")
                    for s in range(4):
                        nc.sync.dma_start(x_sb[:, s, :],
                                          x_d[b0 + s * 128: b0 + (s + 1) * 128, :])
                    xt_sb = xtp.tile([128, 4, MCB], f32, tag="xt")
                    for c, (c0, cw) in enumerate(DIN_CHUNKS):
                        xt_ps = psA.tile([128, 512], f32, tag="psa")
                        for s in range(4):
                            nc.tensor.transpose(xt_ps[:cw, s * 128:(s + 1) * 128],
                                                x_sb[:, s, c0:c0 + cw], ident[:])
                        nc.scalar.copy(xt_sb[:cw, c, :], xt_ps[:cw, :])
                    h_ps = psA.tile([128, 512], f32, tag="psa")
                    for c, (c0, cw) in enumerate(DIN_CHUNKS):
                        nc.tensor.matmul(h_ps[:], encw_sb[:cw, c, :], xt_sb[:cw, c, :],
                                         start=(c == 0), stop=(c == 3))
                    henc = hp.tile([D, MCB], f32, tag="henc")
                    nc.scalar.activation(henc[:], h_ps[:], AF.Relu, bias=encb_sb[:])

                ga = stream(henc[:, 0:SW], b0, 0)
                gb = stream(henc[:, SW:MCB], b0, SW)
                next(ga)
                while True:
                    a_done = b_done = False
                    try:
                        next(ga)
                    except StopIteration:
                        a_done = True
                    try:
                        next(gb)
                    except StopIteration:
                        b_done = True
                    if a_done and b_done:
                        break

            _main_pools.close()

    # Steer the ACT table-set chooser to the one set that contains every
    # function this kernel uses (Exp, Ln, Relu, Copy, Identity).  Without
    # this, Exp picks exp_and_others and each Ln forces a ~1.3us table
    # reload (193 reloads = ~250us wasted on the scalar engine).  Pruning
    # our functions from every other set leaves the chooser a unique
    # choice; dict order (= act_func_set_id mapping) is unchanged.
    from concourse import hw_specs
    import concourse.bacc as bacc_mod
    orig_fn = hw_specs.get_activation_tables
    keep = "natural_log_exp_and_others"
    mine = {AF.Exp, AF.Ln, AF.Relu, AF.Copy, AF.Identity, AF.Square,
            AF.MemsetZero, AF.Abs, AF.Sign, AF.Is_finite, AF.Prelu}

    def patched(arch):
        t = {k: set(v) for k, v in orig_fn(arch).items()}
        assert mine & t[keep] >= {AF.Exp, AF.Ln, AF.Relu, AF.Copy, AF.Identity}
        for k in t:
            if k != keep:
                t[k] = t[k] - mine
        return t

    bacc_mod.get_activation_tables = patched
    hw_specs_orig = hw_specs.get_activation_tables
    hw_specs.get_activation_tables = patched
    try:
        nc.compile()
    finally:
        bacc_mod.get_activation_tables = orig_fn
        hw_specs.get_activation_tables = hw_specs_orig
    return nc


def _prep_host(inputs):
    g = lambda k: np.asarray(inputs[k], dtype=np.float32)
    x = g("x")
    Wq, bq = g("Wq"), g("bq")
    Wk, bk = g("Wk"), g("bk")
    Wv, bv = g("Wv"), g("bv")
    Wo, bo = g("Wo"), g("bo")
    ln1_g, ln1_b = g("ln1_g"), g("ln1_b")
    ln2_g, ln2_b = g("ln2_g"), g("ln2_b")
    W1, b1 = g("W1"), g("b1")
    W2, b2 = g("W2"), g("b2")

    wqkv = np.zeros((NB, D, 3 * D), np.float32)
    bqkv = np.zeros((NB, 1, 3 * D), np.float32)
    w1e = np.zeros((NB, D, FF), np.float32)
    b1e = np.zeros((NB, FF, 1), np.float32)
    for i in range(NB):
        cat_raw = np.concatenate(
            [np.transpose(Wq[i], (1, 0, 2)).reshape(D, D),
             np.transpose(Wk[i], (1, 0, 2)).reshape(D, D),
             np.transpose(Wv[i], (1, 0, 2)).reshape(D, D)], axis=1)   # [D, 384]
        wqkv[i] = ln1_g[i][:, None] * cat_raw
        bqkv[i, 0] = (np.concatenate([bq[i].ravel(), bk[i].ravel(), bv[i].ravel()])
                      + ln1_b[i] @ cat_raw)
        w1e[i] = ln2_g[i][:, None] * W1[i]
        b1e[i, :, 0] = ln2_b[i] @ W1[i] + b1[i]

    arrs = {
        "x": x,
        "encw": g("enc_w"),
        "encb": g("enc_b").reshape(D, 1),
        "wqkv": wqkv,
        "bqkv": bqkv,
        "wo": Wo,
        "bo": bo.reshape(NB, 1, D),
        "w1": w1e,
        "b1": b1e,
        "w2": W2,
        "b2": b2.reshape(NB, 1, D),
        "valw": g("val_w"),
        "valb": g("val_b").reshape(1, 1),
        "polwT": np.ascontiguousarray(g("pol_w").T),
        "polb": g("pol_b").reshape(POL, 1),
        "wmT": np.ascontiguousarray(g("word_matrix").T),
    }
    arrs = {k: np.ascontiguousarray(v, dtype=np.float32) for k, v in arrs.items()}
    skips = (bool(np.all(bqkv == 0.0)), bool(np.all(bo == 0.0)),
             bool(np.all(b2 == 0.0)), bool(np.all(arrs["polb"] == 0.0)),
             bool(np.all(b1e == 0.0)))
    return arrs, skips


def kernel(**inputs):
    from concourse.bass_utils import run_bass_kernel_spmd

    arrs, skips = _prep_host(inputs)
    key = (BC,) + skips
    if key not in _PROGRAM_CACHE:
        _PROGRAM_CACHE[key] = _build_program(BC, *skips)
    nc = _PROGRAM_CACHE[key]

    weights = {k: v for k, v in arrs.items() if k != "x"}
    in_maps = []
    for c in range(NCORES):
        m = dict(weights)
        m["x"] = np.ascontiguousarray(arrs["x"][c * BC:(c + 1) * BC])
        in_maps.append(m)

    res = run_bass_kernel_spmd(nc, in_maps, core_ids=list(range(NCORES)),
                               trace=bool(int(os.environ.get("KERNEL_TRACE", "0"))))
    policy = np.concatenate([r["policy"] for r in res.results], axis=0)
    value = np.concatenate([r["value"] for r in res.results], axis=0)
    if res.exec_time_ns is not None:
        print(f"HW exec time: {res.exec_time_ns} ns")
    kernel.last_results = res
    return policy, value


kernel.last_results = None


# revision 26
# speedup vs baseline: 1.0222x; 1.0222x over previous
"""Trainium2 Bass kernel for nn_ActorCriticTransformer.

Data-parallel over 8 NeuronCores: each core runs the full network on a
4096-row slice of the batch.  Weights are replicated.

Layout strategy per core:
  - Residual stream h kept FEATURE-major [128 d, 512 b] per macro-chunk.
  - x is loaded batch-major and transposed on the PE (4 x [128,~128] chunks).
  - LayerNorm stats via PE ones-matmul (M=1), rsqrt via ACT ln/exp
    (single activation table set for the whole kernel), broadcast back
    across partitions via K=1 PE matmuls.
  - q,k,v produced batch-major [128 b, 384] (lhsT = y feature-major tile).
  - Outer-product attention computed batch-major with stride-0 broadcast
    APs: P[b,(h,i,j)] = q[b,(h,i)]*k[b,(h,j)] (DVE), E = exp(s*P) (ACT),
    segmented tensor_reduce over j for S and O (DVE), softmax division via
    exp(-ln(S)) (ACT).
  - LN gains/biases folded into the adjacent weights on the host.
  - Policy head folds pol_w @ word_matrix.T on-device once, then
    h.T @ polw_eff streams straight from PSUM to HBM via DMA.
"""

import os
import sys

for _p in ("/opt/trn_rl_repo", "/root/.axon_site/_ro/trn_rl_repo"):
    if os.path.isdir(_p) and _p not in sys.path:
        sys.path.insert(0, _p)

import numpy as np

B = 32768
DIN = 417
D = 128
H = 4
DH = 32
NB = 2
V = 2315
POL = 130
FF = 512
EPS = 1e-5
SCALE = DH ** -0.5
NCORES = 8
BC = B // NCORES          # rows per core
MCB = 512                 # macro-chunk batch size
DIN_CHUNKS = [(0, 128), (128, 128), (256, 128), (384, 33)]
V_CHUNKS = [(0, 512), (512, 512), (1024, 512), (1536, 512), (2048, V - 2048)]

_PROGRAM_CACHE = {}


def _build_program(bc, skip_bqkv, skip_bo, skip_b2, skip_polb, skip_b1):
    import concourse.bacc as bacc
    import concourse.tile as tile
    from concourse import mybir
    import concourse.bass as bass
    from concourse.masks import make_identity

    f32 = mybir.dt.float32
    AF = mybir.ActivationFunctionType
    ALU = mybir.AluOpType
    AX = mybir.AxisListType

    nc = bacc.Bacc(None, target_bir_lowering=False, debug=False)

    x_d = nc.dram_tensor("x", [bc, DIN], f32, kind="ExternalInput")
    encw_d = nc.dram_tensor("encw", [DIN, D], f32, kind="ExternalInput")
    encb_d = nc.dram_tensor("encb", [D, 1], f32, kind="ExternalInput")
    wqkv_d = nc.dram_tensor("wqkv", [NB, D, 3 * D], f32, kind="ExternalInput")
    bqkv_d = nc.dram_tensor("bqkv", [NB, 1, 3 * D], f32, kind="ExternalInput")
    wo_d = nc.dram_tensor("wo", [NB, D, D], f32, kind="ExternalInput")
    bo_d = nc.dram_tensor("bo", [NB, 1, D], f32, kind="ExternalInput")
    w1_d = nc.dram_tensor("w1", [NB, D, FF], f32, kind="ExternalInput")
    b1_d = nc.dram_tensor("b1", [NB, FF, 1], f32, kind="ExternalInput")
    w2_d = nc.dram_tensor("w2", [NB, FF, D], f32, kind="ExternalInput")
    b2_d = nc.dram_tensor("b2", [NB, 1, D], f32, kind="ExternalInput")
    valw_d = nc.dram_tensor("valw", [D, 1], f32, kind="ExternalInput")
    valb_d = nc.dram_tensor("valb", [1, 1], f32, kind="ExternalInput")
    polwT_d = nc.dram_tensor("polwT", [POL, D], f32, kind="ExternalInput")
    polb_d = nc.dram_tensor("polb", [POL, 1], f32, kind="ExternalInput")
    wmT_d = nc.dram_tensor("wmT", [POL, V], f32, kind="ExternalInput")

    policy_d = nc.dram_tensor("policy", [bc, V], f32, kind="ExternalOutput")
    value_d = nc.dram_tensor("value", [bc, 1], f32, kind="ExternalOutput")

    n_mc = bc // MCB

    from contextlib import ExitStack
    with tile.TileContext(nc) as tc:
        with (
            tc.tile_pool(name="singles", bufs=1) as singles,
            tc.tile_pool(name="psA", bufs=2, space="PSUM") as psA,
            tc.tile_pool(name="psB", bufs=2, space="PSUM") as psB,
            tc.tile_pool(name="psC", bufs=2, space="PSUM") as psC,
            tc.tile_pool(name="psD", bufs=2, space="PSUM") as psD,
        ):
            # ---------------- one-time: weights to SBUF ----------------
            # (deprioritized so the first macro-chunk's x-DMA/encoder isn't
            # stuck behind ~2MB of weight DMA + the policy fold)
            _depri = tc.high_priority(offset=-300)
            _depri.__enter__()
            encw_sb = singles.tile([128, 4, D], f32)
            for c, (c0, cw) in enumerate(DIN_CHUNKS):
                nc.sync.dma_start(encw_sb[:cw, c, :], encw_d[c0:c0 + cw, :])
            encb_sb = singles.tile([D, 1], f32)
            nc.sync.dma_start(encb_sb[:], encb_d[:])
            wqkv_sb = singles.tile([D, NB, 3 * D], f32)
            wo_sb = singles.tile([D, NB, D], f32)
            w1_sb = singles.tile([D, NB, FF], f32)
            w2_sb = singles.tile([D, NB, 4, D], f32)
            b1_sb = singles.tile([D, NB, 4, 1], f32)
            for i in range(NB):
                nc.sync.dma_start(wqkv_sb[:, i, :], wqkv_d[i])
                nc.sync.dma_start(wo_sb[:, i, :], wo_d[i])
                nc.sync.dma_start(w1_sb[:, i, :], w1_d[i])
                for c in range(4):
                    nc.sync.dma_start(w2_sb[:, i, c, :], w2_d[i, c * 128:(c + 1) * 128, :])
                    nc.sync.dma_start(b1_sb[:, i, c, :], b1_d[i, c * 128:(c + 1) * 128, :])
            if not skip_bqkv:
                bqkv_sb = singles.tile([1, NB, 3 * D], f32)
                for i in range(NB):
                    nc.sync.dma_start(bqkv_sb[:, i, :], bqkv_d[i])
            if not skip_bo:
                bo_sb = singles.tile([1, NB, D], f32)
                for i in range(NB):
                    nc.sync.dma_start(bo_sb[:, i, :], bo_d[i])
            if not skip_b2:
                b2_sb = singles.tile([1, NB, D], f32)
                for i in range(NB):
                    nc.sync.dma_start(b2_sb[:, i, :], b2_d[i])
            valw_sb = singles.tile([D, 1], f32)
            nc.sync.dma_start(valw_sb[:], valw_d[:])
            valb_sb = singles.tile([1, 1], f32)
            nc.sync.dma_start(valb_sb[:], valb_d[:])

            ident = singles.tile([128, 128], f32)
            with tc.high_priority():
                make_identity(nc, ident[:])
            ones_row = singles.tile([1, 128], f32)
            nc.vector.memset(ones_row[:], 1.0)
            ones512 = singles.tile([1, MCB], f32)
            nc.vector.memset(ones512[:], 1.0)
            inv_col = singles.tile([128, 1], f32)
            nc.vector.memset(inv_col[:], 1.0 / D)
            zero_col = singles.tile([128, 1], f32)
            nc.vector.memset(zero_col[:], 0.0)
            zero1 = singles.tile([1, 1], f32)
            nc.vector.memset(zero1[:], 0.0)
            eps1 = singles.tile([1, 1], f32)
            nc.vector.memset(eps1[:], EPS)

            # ---------------- one-time: policy weight fold ----------------
            # The word-matrix tiles are only needed here; a scoped pool gives
            # the ~20KB/partition back to the main loop afterwards.
            polw_eff = singles.tile([128, V], f32)
            if not skip_polb:
                polb_eff = singles.tile([1, V], f32)
            with tc.tile_pool(name="foldp", bufs=1) as foldp:
                polwT_sb = foldp.tile([128, 2, D], f32)
                nc.sync.dma_start(polwT_sb[:, 0, :], polwT_d[0:128, :])
                nc.sync.dma_start(polwT_sb[:POL - 128, 1, :], polwT_d[128:POL, :])
                wmT_sb = foldp.tile([128, 2, V], f32)
                nc.sync.dma_start(wmT_sb[:, 0, :], wmT_d[0:128, :])
                nc.sync.dma_start(wmT_sb[:POL - 128, 1, :], wmT_d[128:POL, :])
                if not skip_polb:
                    polb_sb = foldp.tile([128, 2, 1], f32)
                    nc.sync.dma_start(polb_sb[:, 0, :], polb_d[0:128, :])
                    nc.sync.dma_start(polb_sb[:POL - 128, 1, :], polb_d[128:POL, :])
                for n0, nw in V_CHUNKS:
                    pw_ps = psA.tile([128, 512], f32, tag="psa")
                    nc.tensor.matmul(pw_ps[:, :nw], polwT_sb[:, 0, :],
                                     wmT_sb[:, 0, n0:n0 + nw], start=True, stop=False)
                    nc.tensor.matmul(pw_ps[:, :nw], polwT_sb[:POL - 128, 1, :],
                                     wmT_sb[:POL - 128, 1, n0:n0 + nw], start=False, stop=True)
                    nc.scalar.copy(polw_eff[:, n0:n0 + nw], pw_ps[:, :nw])
                    if not skip_polb:
                        pb_ps = psC.tile([1, 512], f32, tag="psc")
                        nc.tensor.matmul(pb_ps[:, :nw], polb_sb[:, 0, :],
                                         wmT_sb[:, 0, n0:n0 + nw], start=True, stop=False)
                        nc.tensor.matmul(pb_ps[:, :nw], polb_sb[:POL - 128, 1, :],
                                         wmT_sb[:POL - 128, 1, n0:n0 + nw], start=False, stop=True)
                        nc.vector.tensor_copy(polb_eff[:, n0:n0 + nw], pb_ps[:, :nw])

            _depri.__exit__(None, None, None)

            _main_pools = ExitStack()
            xp = _main_pools.enter_context(tc.tile_pool(name="xp", bufs=2))
            xtp = _main_pools.enter_context(tc.tile_pool(name="xtp", bufs=1))
            hp = _main_pools.enter_context(tc.tile_pool(name="hp", bufs=3))
            lnp = _main_pools.enter_context(tc.tile_pool(name="lnp", bufs=2))
            lns = _main_pools.enter_context(tc.tile_pool(name="lns", bufs=2))
            yp = _main_pools.enter_context(tc.tile_pool(name="yp", bufs=2))
            qp = _main_pools.enter_context(tc.tile_pool(name="qp", bufs=4))
            bigp = _main_pools.enter_context(tc.tile_pool(name="bigp", bufs=2))
            bige = _main_pools.enter_context(tc.tile_pool(name="bige", bufs=2))
            smp = _main_pools.enter_context(tc.tile_pool(name="smp", bufs=3))
            ofp = _main_pools.enter_context(tc.tile_pool(name="ofp", bufs=2))
            ffp = _main_pools.enter_context(tc.tile_pool(name="ffp", bufs=2))

            # ---------------- layernorm (feature-major, width-parametric) -------
            def layernorm(h_ap, W):
                hsq = lnp.tile([D, W], f32, tag="hsq")
                nc.vector.tensor_mul(hsq[:], h_ap, h_ap)
                mu_ps = psC.tile([1, W], f32, tag="psc")
                m2_ps = psC.tile([1, W], f32, tag="psc")
                nc.tensor.matmul(mu_ps[:], inv_col[:], h_ap, start=True, stop=True)
                nc.tensor.matmul(m2_ps[:], inv_col[:], hsq[:], start=True, stop=True)
                mu = lns.tile([1, W], f32, tag="mu")
                nc.scalar.copy(mu[:], mu_ps[:])
                # var = m2 - mu^2 ; rstd = exp(-0.5*ln(var+eps))
                musq = lns.tile([1, W], f32, tag="musq")
                nc.scalar.activation(musq[:], mu_ps[:], AF.Square, bias=zero1[:])
                var = lns.tile([1, W], f32, tag="var")
                nc.vector.tensor_sub(var[:], m2_ps[:], musq[:])
                lnv = lns.tile([1, W], f32, tag="lnv")
                nc.scalar.activation(lnv[:], var[:], AF.Ln, bias=eps1[:])
                rstd = lns.tile([1, W], f32, tag="rstd")
                nc.scalar.activation(rstd[:], lnv[:], AF.Exp, bias=zero1[:], scale=-0.5)
                mu_bc = psB.tile([128, W], f32, tag="psb")
                rs_bc = psB.tile([128, W], f32, tag="psb")
                nc.tensor.matmul(mu_bc[:], ones_row[:], mu[:], start=True, stop=True)
                nc.tensor.matmul(rs_bc[:], ones_row[:], rstd[:], start=True, stop=True)
                t = lnp.tile([D, W], f32, tag="t")
                nc.vector.tensor_sub(t[:], h_ap, mu_bc[:])
                y = yp.tile([D, W], f32, tag="y")
                nc.vector.tensor_mul(y[:], t[:], rs_bc[:])
                return y

            # ---------------- attention stages (per 128-sample subtile) --------
            def stage_a(y, loc, blk):
                qkv_ps = psB.tile([128, 3 * D], f32, tag="psb")
                nc.tensor.matmul(qkv_ps[:], y[:, loc * 128:(loc + 1) * 128],
                                 wqkv_sb[:, blk, :], start=True, stop=skip_bqkv)
                if not skip_bqkv:
                    nc.tensor.matmul(qkv_ps[:], ones_row[:], bqkv_sb[:, blk, :],
                                     start=False, stop=True)
                qkv = qp.tile([128, 3 * D], f32, tag="qkv")
                nc.scalar.copy(qkv[:], qkv_ps[:])
                q_ap = qkv[:, 0:D]
                k_ap = qkv[:, D:2 * D]
                q_bc = bass.AP(tensor=q_ap.tensor, offset=q_ap.offset,
                               ap=[q_ap.ap[0], [DH, H], [1, DH], [0, DH]])
                k_bc = bass.AP(tensor=k_ap.tensor, offset=k_ap.offset,
                               ap=[k_ap.ap[0], [DH, H], [0, DH], [1, DH]])
                P = bigp.tile([128, H * DH * DH], f32, tag="P")
                nc.vector.tensor_mul(
                    P[:].rearrange("p (h i j) -> p h i j", h=H, i=DH), q_bc, k_bc)
                E = bige.tile([128, H * DH * DH], f32, tag="E")
                nc.scalar.activation(E[:], P[:], AF.Exp, bias=zero_col[:], scale=SCALE)
                return {"qkv": qkv, "P": P, "E": E}

            def stage_b(st):
                S = smp.tile([128, D], f32, tag="S")
                nc.vector.tensor_reduce(
                    S[:], st["E"][:].rearrange("p (a j) -> p a j", j=DH),
                    axis=AX.X, op=ALU.add)
                v_ap = st["qkv"][:, 2 * D:3 * D]
                v_bc = bass.AP(tensor=v_ap.tensor, offset=v_ap.offset,
                               ap=[v_ap.ap[0], [DH, H], [0, DH], [1, DH]])
                # EV into P's slot (dead after exp); GPSIMD is otherwise idle.
                nc.gpsimd.tensor_mul(
                    st["P"][:].rearrange("p (h i j) -> p h i j", h=H, i=DH),
                    st["E"][:].rearrange("p (h i j) -> p h i j", h=H, i=DH), v_bc)
                lnS = smp.tile([128, D], f32, tag="lnS")
                nc.scalar.activation(lnS[:], S[:], AF.Ln, bias=zero_col[:])
                rS = smp.tile([128, D], f32, tag="rS")
                nc.scalar.activation(rS[:], lnS[:], AF.Exp, bias=zero_col[:], scale=-1.0)
                st["rS"] = rS

            def stage_c(st, ofm, loc):
                O = smp.tile([128, D], f32, tag="O")
                nc.vector.tensor_reduce(
                    O[:], st["P"][:].rearrange("p (a j) -> p a j", j=DH),
                    axis=AX.X, op=ALU.add)
                On = smp.tile([128, D], f32, tag="On")
                nc.vector.tensor_mul(On[:], O[:], st["rS"][:])
                ot_ps = psC.tile([128, D], f32, tag="psc")
                nc.tensor.transpose(ot_ps[:], On[:], ident[:])
                nc.scalar.copy(ofm[:, loc * 128:(loc + 1) * 128], ot_ps[:])

            # ---------------- per-macro-chunk network body ----------------
            def stream(h0_ap, b0):
                h_ap = h0_ap
                for blk in range(NB):
                    y = layernorm(h_ap, MCB)
                    ofm = ofp.tile([D, MCB], f32, tag="ofm")
                    sts = {}
                    for pair in ((0, 1), (2, 3)):
                        for s in pair:
                            sts[s] = stage_a(y, s, blk)
                        for s in pair:
                            stage_b(sts[s])
                        for s in pair:
                            stage_c(sts[s], ofm, s)
                    wo_ps = psA.tile([128, MCB], f32, tag="psa")
                    nc.tensor.matmul(wo_ps[:], wo_sb[:, blk, :], ofm[:],
                                     start=True, stop=skip_bo)
                    if not skip_bo:
                        nc.tensor.matmul(wo_ps[:], bo_sb[:, blk, :], ones512[:],
                                         start=False, stop=True)
                    h2 = hp.tile([D, MCB], f32, tag="h")
                    nc.vector.tensor_add(h2[:], h_ap, wo_ps[:])
                    h_ap = h2[:]
                    y2 = layernorm(h_ap, MCB)
                    ffs = ffp.tile([128, 4, MCB], f32, tag="ffs")
                    w2_ps = psA.tile([128, MCB], f32, tag="psa")
                    for c in range(4):
                        ff_ps = psB.tile([128, MCB], f32, tag="psb")
                        nc.tensor.matmul(ff_ps[:], w1_sb[:, blk, c * 128:(c + 1) * 128],
                                         y2[:], start=True, stop=True)
                        if skip_b1:
                            nc.vector.tensor_relu(ffs[:, c, :], ff_ps[:])
                        else:
                            nc.scalar.activation(ffs[:, c, :], ff_ps[:], AF.Relu,
                                                 bias=b1_sb[:, blk, c, :])
                        nc.tensor.matmul(w2_ps[:], w2_sb[:, blk, c, :], ffs[:, c, :],
                                         start=(c == 0), stop=(c == 3 and skip_b2))
                    if not skip_b2:
                        nc.tensor.matmul(w2_ps[:], b2_sb[:, blk, :], ones512[:],
                                         start=False, stop=True)
                    h3 = hp.tile([D, MCB], f32, tag="h")
                    nc.vector.tensor_add(h3[:], h_ap, w2_ps[:])
                    h_ap = h3[:]
                # heads
                val_ps = psC.tile([1, MCB], f32, tag="psc")
                nc.tensor.matmul(val_ps[:], valw_sb[:], h_ap, start=True, stop=True)
                val_sb = lns.tile([1, MCB], f32, tag="val")
                nc.scalar.activation(val_sb[:], val_ps[:], AF.Identity, bias=valb_sb[:])
                nc.sync.dma_start(value_d[b0:b0 + MCB, :], val_sb[:])
                with tc.high_priority(offset=-150):
                    for s in range(4):
                        for n0, nw in V_CHUNKS:
                            pol_ps = psD.tile([128, 512], f32, tag="psd")
                            nc.tensor.matmul(pol_ps[:, :nw],
                                             h_ap[:, s * 128:(s + 1) * 128],
                                             polw_eff[:, n0:n0 + nw], start=True,
                                             stop=skip_polb)
                            if not skip_polb:
                                nc.tensor.matmul(pol_ps[:, :nw], ones_row[:],
                                                 polb_eff[:, n0:n0 + nw], start=False,
                                                 stop=True)
                            pol_sb = ofp.tile([128, 512], f32, tag="pol")
                            nc.scalar.copy(pol_sb[:, :nw], pol_ps[:, :nw])
                            nc.sync.dma_start(
                                policy_d[b0 + s * 128: b0 + (s + 1) * 128, n0:n0 + nw],
                                pol_sb[:, :nw])

            # ---------------- main loop ----------------
            for mc in range(n_mc):
                b0 = mc * MCB
                # Emit the load/transpose/encode front-end "earlier" so it fills
                # the previous macro-chunk's FF/LN engine bubbles.
                with tc.high_priority(offset=(250 if mc > 0 else None)):
                    x_sb = xp.tile([128, 4, DIN], f32, tag="x")
                    for s in range(4):
                        nc.sync.dma_start(x_sb[:, s, :],
                                          x_d[b0 + s * 128: b0 + (s + 1) * 128, :])
                    xt_sb = xtp.tile([128, 4, MCB], f32, tag="xt")
                    for c, (c0, cw) in enumerate(DIN_CHUNKS):
                        xt_ps = psA.tile([128, 512], f32, tag="psa")
                        for s in range(4):
                            nc.tensor.transpose(xt_ps[:cw, s * 128:(s + 1) * 128],
                                                x_sb[:, s, c0:c0 + cw], ident[:])
                        nc.scalar.copy(xt_sb[:cw, c, :], xt_ps[:cw, :])
                    h_ps = psA.tile([128, 512], f32, tag="psa")
                    for c, (c0, cw) in enumerate(DIN_CHUNKS):
                        nc.tensor.matmul(h_ps[:], encw_sb[:cw, c, :], xt_sb[:cw, c, :],
                                         start=(c == 0), stop=(c == 3))
                    henc = hp.tile([D, MCB], f32, tag="henc")
                    nc.scalar.activation(henc[:], h_ps[:], AF.Relu, bias=encb_sb[:])

                stream(henc[:], b0)

            _main_pools.close()

    # Steer the ACT table-set chooser to the one set that contains every
    # function this kernel uses (Exp, Ln, Relu, Copy, Identity).  Without
    # this, Exp picks exp_and_others and each Ln forces a ~1.3us table
    # reload (193 reloads = ~250us wasted on the scalar engine).  Pruning
    # our functions from every other set leaves the chooser a unique
    # choice; dict order (= act_func_set_id mapping) is unchanged.
    from concourse import hw_specs
    import concourse.bacc as bacc_mod
    orig_fn = hw_specs.get_activation_tables
    keep = "natural_log_exp_and_others"
    mine = {AF.Exp, AF.Ln, AF.Relu, AF.Copy, AF.Identity, AF.Square,
            AF.MemsetZero, AF.Abs, AF.Sign, AF.Is_finite, AF.Prelu}

    def patched(arch):
        t = {k: set(v) for k, v in orig_fn(arch).items()}
        assert mine & t[keep] >= {AF.Exp, AF.Ln, AF.Relu, AF.Copy, AF.Identity}
        for k in t:
            if k != keep:
                t[k] = t[k] - mine
        return t

    bacc_mod.get_activation_tables = patched
    hw_specs_orig = hw_specs.get_activation_tables
    hw_specs.get_activation_tables = patched
    try:
        nc.compile()
    finally:
        bacc_mod.get_activation_tables = orig_fn
        hw_specs.get_activation_tables = hw_specs_orig
    return nc


def _prep_host(inputs):
    g = lambda k: np.asarray(inputs[k], dtype=np.float32)
    x = g("x")
    Wq, bq = g("Wq"), g("bq")
    Wk, bk = g("Wk"), g("bk")
    Wv, bv = g("Wv"), g("bv")
    Wo, bo = g("Wo"), g("bo")
    ln1_g, ln1_b = g("ln1_g"), g("ln1_b")
    ln2_g, ln2_b = g("ln2_g"), g("ln2_b")
    W1, b1 = g("W1"), g("b1")
    W2, b2 = g("W2"), g("b2")

    wqkv = np.zeros((NB, D, 3 * D), np.float32)
    bqkv = np.zeros((NB, 1, 3 * D), np.float32)
    w1e = np.zeros((NB, D, FF), np.float32)
    b1e = np.zeros((NB, FF, 1), np.float32)
    for i in range(NB):
        cat_raw = np.concatenate(
            [np.transpose(Wq[i], (1, 0, 2)).reshape(D, D),
             np.transpose(Wk[i], (1, 0, 2)).reshape(D, D),
             np.transpose(Wv[i], (1, 0, 2)).reshape(D, D)], axis=1)   # [D, 384]
        wqkv[i] = ln1_g[i][:, None] * cat_raw
        bqkv[i, 0] = (np.concatenate([bq[i].ravel(), bk[i].ravel(), bv[i].ravel()])
                      + ln1_b[i] @ cat_raw)
        w1e[i] = ln2_g[i][:, None] * W1[i]
        b1e[i, :, 0] = ln2_b[i] @ W1[i] + b1[i]

    arrs = {
        "x": x,
        "encw": g("enc_w"),
        "encb": g("enc_b").reshape(D, 1),
        "wqkv": wqkv,
        "bqkv": bqkv,
        "wo": Wo,
        "bo": bo.reshape(NB, 1, D),
        "w1": w1e,
        "b1": b1e,
        "w2": W2,
        "b2": b2.reshape(NB, 1, D),
        "valw": g("val_w"),
        "valb": g("val_b").reshape(1, 1),
        "polwT": np.ascontiguousarray(g("pol_w").T),
        "polb": g("pol_b").reshape(POL, 1),
        "wmT": np.ascontiguousarray(g("word_matrix").T),
    }
    arrs = {k: np.ascontiguousarray(v, dtype=np.float32) for k, v in arrs.items()}
    skips = (bool(np.all(bqkv == 0.0)), bool(np.all(bo == 0.0)),
             bool(np.all(b2 == 0.0)), bool(np.all(arrs["polb"] == 0.0)),
             bool(np.all(b1e == 0.0)))
    return arrs, skips


def kernel(**inputs):
    from concourse.bass_utils import run_bass_kernel_spmd

    arrs, skips = _prep_host(inputs)
    key = (BC,) + skips
    if key not in _PROGRAM_CACHE:
        _PROGRAM_CACHE[key] = _build_program(BC, *skips)
    nc = _PROGRAM_CACHE[key]

    weights = {k: v for k, v in arrs.items() if k != "x"}
    in_maps = []
    for c in range(NCORES):
        m = dict(weights)
        m["x"] = np.ascontiguousarray(arrs["x"][c * BC:(c + 1) * BC])
        in_maps.append(m)

    res = run_bass_kernel_spmd(nc, in_maps, core_ids=list(range(NCORES)),
                               trace=bool(int(os.environ.get("KERNEL_TRACE", "0"))))
    policy = np.concatenate([r["policy"] for r in res.results], axis=0)
    value = np.concatenate([r["value"] for r in res.results], axis=0)
    if res.exec_time_ns is not None:
        print(f"HW exec time: {res.exec_time_ns} ns")
    kernel.last_results = res
    return policy, value


kernel.last_results = None


# revision 27
# speedup vs baseline: 1.0369x; 1.0144x over previous
"""Trainium2 Bass kernel for nn_ActorCriticTransformer.

Data-parallel over 8 NeuronCores: each core runs the full network on a
4096-row slice of the batch.  Weights are replicated.

Layout strategy per core:
  - Residual stream h kept FEATURE-major [128 d, 512 b] per macro-chunk.
  - x is loaded batch-major and transposed on the PE (4 x [128,~128] chunks).
  - LayerNorm stats via PE ones-matmul (M=1), rsqrt via ACT ln/exp
    (single activation table set for the whole kernel), broadcast back
    across partitions via K=1 PE matmuls.
  - q,k,v produced batch-major [128 b, 384] (lhsT = y feature-major tile).
  - Outer-product attention computed batch-major with stride-0 broadcast
    APs: P[b,(h,i,j)] = q[b,(h,i)]*k[b,(h,j)] (DVE), E = exp(s*P) (ACT),
    segmented tensor_reduce over j for S and O (DVE), softmax division via
    exp(-ln(S)) (ACT).
  - LN gains/biases folded into the adjacent weights on the host.
  - Policy head folds pol_w @ word_matrix.T on-device once, then
    h.T @ polw_eff streams straight from PSUM to HBM via DMA.
"""

import os
import sys

for _p in ("/opt/trn_rl_repo", "/root/.axon_site/_ro/trn_rl_repo"):
    if os.path.isdir(_p) and _p not in sys.path:
        sys.path.insert(0, _p)

import numpy as np

B = 32768
DIN = 417
D = 128
H = 4
DH = 32
NB = 2
V = 2315
POL = 130
FF = 512
EPS = 1e-5
SCALE = DH ** -0.5
NCORES = 8
BC = B // NCORES          # rows per core
MCB = 512                 # macro-chunk batch size
DIN_CHUNKS = [(0, 128), (128, 128), (256, 128), (384, 33)]
V_CHUNKS = [(0, 512), (512, 512), (1024, 512), (1536, 512), (2048, V - 2048)]

_PROGRAM_CACHE = {}


def _build_program(bc, skip_bqkv, skip_bo, skip_b2, skip_polb, skip_b1):
    import concourse.bacc as bacc
    import concourse.tile as tile
    from concourse import mybir
    import concourse.bass as bass
    from concourse.masks import make_identity

    f32 = mybir.dt.float32
    AF = mybir.ActivationFunctionType
    ALU = mybir.AluOpType
    AX = mybir.AxisListType

    nc = bacc.Bacc(None, target_bir_lowering=False, debug=False)

    x_d = nc.dram_tensor("x", [bc, DIN], f32, kind="ExternalInput")
    encw_d = nc.dram_tensor("encw", [DIN, D], f32, kind="ExternalInput")
    encb_d = nc.dram_tensor("encb", [D, 1], f32, kind="ExternalInput")
    wqkv_d = nc.dram_tensor("wqkv", [NB, D, 3 * D], f32, kind="ExternalInput")
    bqkv_d = nc.dram_tensor("bqkv", [NB, 1, 3 * D], f32, kind="ExternalInput")
    wo_d = nc.dram_tensor("wo", [NB, D, D], f32, kind="ExternalInput")
    bo_d = nc.dram_tensor("bo", [NB, 1, D], f32, kind="ExternalInput")
    w1_d = nc.dram_tensor("w1", [NB, D, FF], f32, kind="ExternalInput")
    b1_d = nc.dram_tensor("b1", [NB, FF, 1], f32, kind="ExternalInput")
    w2_d = nc.dram_tensor("w2", [NB, FF, D], f32, kind="ExternalInput")
    b2_d = nc.dram_tensor("b2", [NB, 1, D], f32, kind="ExternalInput")
    valw_d = nc.dram_tensor("valw", [D, 1], f32, kind="ExternalInput")
    valb_d = nc.dram_tensor("valb", [1, 1], f32, kind="ExternalInput")
    polwT_d = nc.dram_tensor("polwT", [POL, D], f32, kind="ExternalInput")
    polb_d = nc.dram_tensor("polb", [POL, 1], f32, kind="ExternalInput")
    wmT_d = nc.dram_tensor("wmT", [POL, V], f32, kind="ExternalInput")

    policy_d = nc.dram_tensor("policy", [bc, V], f32, kind="ExternalOutput")
    value_d = nc.dram_tensor("value", [bc, 1], f32, kind="ExternalOutput")

    n_mc = bc // MCB

    from contextlib import ExitStack
    with tile.TileContext(nc) as tc:
        with (
            tc.tile_pool(name="singles", bufs=1) as singles,
            tc.tile_pool(name="psA", bufs=2, space="PSUM") as psA,
            tc.tile_pool(name="psB", bufs=2, space="PSUM") as psB,
            tc.tile_pool(name="psC", bufs=2, space="PSUM") as psC,
            tc.tile_pool(name="psD", bufs=2, space="PSUM") as psD,
        ):
            # ---------------- one-time: weights to SBUF ----------------
            # (deprioritized so the first macro-chunk's x-DMA/encoder isn't
            # stuck behind ~2MB of weight DMA + the policy fold)
            _depri = tc.high_priority(offset=-300)
            _depri.__enter__()
            encw_sb = singles.tile([128, 4, D], f32)
            for c, (c0, cw) in enumerate(DIN_CHUNKS):
                nc.sync.dma_start(encw_sb[:cw, c, :], encw_d[c0:c0 + cw, :])
            encb_sb = singles.tile([D, 1], f32)
            nc.sync.dma_start(encb_sb[:], encb_d[:])
            wqkv_sb = singles.tile([D, NB, 3 * D], f32)
            wo_sb = singles.tile([D, NB, D], f32)
            w1_sb = singles.tile([D, NB, FF], f32)
            w2_sb = singles.tile([D, NB, 4, D], f32)
            b1_sb = singles.tile([D, NB, 4, 1], f32)
            for i in range(NB):
                nc.sync.dma_start(wqkv_sb[:, i, :], wqkv_d[i])
                nc.sync.dma_start(wo_sb[:, i, :], wo_d[i])
                nc.sync.dma_start(w1_sb[:, i, :], w1_d[i])
                for c in range(4):
                    nc.sync.dma_start(w2_sb[:, i, c, :], w2_d[i, c * 128:(c + 1) * 128, :])
                    nc.sync.dma_start(b1_sb[:, i, c, :], b1_d[i, c * 128:(c + 1) * 128, :])
            if not skip_bqkv:
                bqkv_sb = singles.tile([1, NB, 3 * D], f32)
                for i in range(NB):
                    nc.sync.dma_start(bqkv_sb[:, i, :], bqkv_d[i])
            if not skip_bo:
                bo_sb = singles.tile([1, NB, D], f32)
                for i in range(NB):
                    nc.sync.dma_start(bo_sb[:, i, :], bo_d[i])
            if not skip_b2:
                b2_sb = singles.tile([1, NB, D], f32)
                for i in range(NB):
                    nc.sync.dma_start(b2_sb[:, i, :], b2_d[i])
            valw_sb = singles.tile([D, 1], f32)
            nc.sync.dma_start(valw_sb[:], valw_d[:])
            valb_sb = singles.tile([1, 1], f32)
            nc.sync.dma_start(valb_sb[:], valb_d[:])

            ident = singles.tile([128, 128], f32)
            with tc.high_priority():
                make_identity(nc, ident[:])
            ones_row = singles.tile([1, 128], f32)
            nc.vector.memset(ones_row[:], 1.0)
            ones512 = singles.tile([1, MCB], f32)
            nc.vector.memset(ones512[:], 1.0)
            inv_col = singles.tile([128, 1], f32)
            nc.vector.memset(inv_col[:], 1.0 / D)
            zero_col = singles.tile([128, 1], f32)
            nc.vector.memset(zero_col[:], 0.0)
            zero1 = singles.tile([1, 1], f32)
            nc.vector.memset(zero1[:], 0.0)
            eps1 = singles.tile([1, 1], f32)
            nc.vector.memset(eps1[:], EPS)

            # ---------------- one-time: policy weight fold ----------------
            # The word-matrix tiles are only needed here; a scoped pool gives
            # the ~20KB/partition back to the main loop afterwards.
            polw_eff = singles.tile([128, V], f32)
            if not skip_polb:
                polb_eff = singles.tile([1, V], f32)
            with tc.tile_pool(name="foldp", bufs=1) as foldp:
                polwT_sb = foldp.tile([128, 2, D], f32)
                nc.sync.dma_start(polwT_sb[:, 0, :], polwT_d[0:128, :])
                nc.sync.dma_start(polwT_sb[:POL - 128, 1, :], polwT_d[128:POL, :])
                wmT_sb = foldp.tile([128, 2, V], f32)
                nc.sync.dma_start(wmT_sb[:, 0, :], wmT_d[0:128, :])
                nc.sync.dma_start(wmT_sb[:POL - 128, 1, :], wmT_d[128:POL, :])
                if not skip_polb:
                    polb_sb = foldp.tile([128, 2, 1], f32)
                    nc.sync.dma_start(polb_sb[:, 0, :], polb_d[0:128, :])
                    nc.sync.dma_start(polb_sb[:POL - 128, 1, :], polb_d[128:POL, :])
                for n0, nw in V_CHUNKS:
                    pw_ps = psA.tile([128, 512], f32, tag="psa")
                    nc.tensor.matmul(pw_ps[:, :nw], polwT_sb[:, 0, :],
                                     wmT_sb[:, 0, n0:n0 + nw], start=True, stop=False)
                    nc.tensor.matmul(pw_ps[:, :nw], polwT_sb[:POL - 128, 1, :],
                                     wmT_sb[:POL - 128, 1, n0:n0 + nw], start=False, stop=True)
                    nc.scalar.copy(polw_eff[:, n0:n0 + nw], pw_ps[:, :nw])
                    if not skip_polb:
                        pb_ps = psC.tile([1, 512], f32, tag="psc")
                        nc.tensor.matmul(pb_ps[:, :nw], polb_sb[:, 0, :],
                                         wmT_sb[:, 0, n0:n0 + nw], start=True, stop=False)
                        nc.tensor.matmul(pb_ps[:, :nw], polb_sb[:POL - 128, 1, :],
                                         wmT_sb[:POL - 128, 1, n0:n0 + nw], start=False, stop=True)
                        nc.vector.tensor_copy(polb_eff[:, n0:n0 + nw], pb_ps[:, :nw])

            _depri.__exit__(None, None, None)

            _main_pools = ExitStack()
            xp = _main_pools.enter_context(tc.tile_pool(name="xp", bufs=1))
            xtp = _main_pools.enter_context(tc.tile_pool(name="xtp", bufs=1))
            hp = _main_pools.enter_context(tc.tile_pool(name="hp", bufs=3))
            lnp = _main_pools.enter_context(tc.tile_pool(name="lnp", bufs=2))
            lns = _main_pools.enter_context(tc.tile_pool(name="lns", bufs=2))
            yp = _main_pools.enter_context(tc.tile_pool(name="yp", bufs=2))
            qp = _main_pools.enter_context(tc.tile_pool(name="qp", bufs=4))
            bigp = _main_pools.enter_context(tc.tile_pool(name="bigp", bufs=3))
            bige = _main_pools.enter_context(tc.tile_pool(name="bige", bufs=2))
            smp = _main_pools.enter_context(tc.tile_pool(name="smp", bufs=3))
            ofp = _main_pools.enter_context(tc.tile_pool(name="ofp", bufs=2))
            ffp = _main_pools.enter_context(tc.tile_pool(name="ffp", bufs=1))

            # ---------------- layernorm (feature-major, width-parametric) -------
            def layernorm(h_ap, W):
                hsq = lnp.tile([D, W], f32, tag="hsq")
                nc.vector.tensor_mul(hsq[:], h_ap, h_ap)
                mu_ps = psC.tile([1, W], f32, tag="psc")
                m2_ps = psC.tile([1, W], f32, tag="psc")
                nc.tensor.matmul(mu_ps[:], inv_col[:], h_ap, start=True, stop=True)
                nc.tensor.matmul(m2_ps[:], inv_col[:], hsq[:], start=True, stop=True)
                mu = lns.tile([1, W], f32, tag="mu")
                nc.scalar.copy(mu[:], mu_ps[:])
                # var = m2 - mu^2 ; rstd = exp(-0.5*ln(var+eps))
                musq = lns.tile([1, W], f32, tag="musq")
                nc.scalar.activation(musq[:], mu_ps[:], AF.Square, bias=zero1[:])
                var = lns.tile([1, W], f32, tag="var")
                nc.vector.tensor_sub(var[:], m2_ps[:], musq[:])
                lnv = lns.tile([1, W], f32, tag="lnv")
                nc.scalar.activation(lnv[:], var[:], AF.Ln, bias=eps1[:])
                rstd = lns.tile([1, W], f32, tag="rstd")
                nc.scalar.activation(rstd[:], lnv[:], AF.Exp, bias=zero1[:], scale=-0.5)
                mu_bc = psB.tile([128, W], f32, tag="psb")
                rs_bc = psB.tile([128, W], f32, tag="psb")
                nc.tensor.matmul(mu_bc[:], ones_row[:], mu[:], start=True, stop=True)
                nc.tensor.matmul(rs_bc[:], ones_row[:], rstd[:], start=True, stop=True)
                t = lnp.tile([D, W], f32, tag="t")
                nc.vector.tensor_sub(t[:], h_ap, mu_bc[:])
                y = yp.tile([D, W], f32, tag="y")
                nc.vector.tensor_mul(y[:], t[:], rs_bc[:])
                return y

            # ---------------- attention stages (per 128-sample subtile) --------
            def stage_a(y, loc, blk):
                qkv_ps = psB.tile([128, 3 * D], f32, tag="psb")
                nc.tensor.matmul(qkv_ps[:], y[:, loc * 128:(loc + 1) * 128],
                                 wqkv_sb[:, blk, :], start=True, stop=skip_bqkv)
                if not skip_bqkv:
                    nc.tensor.matmul(qkv_ps[:], ones_row[:], bqkv_sb[:, blk, :],
                                     start=False, stop=True)
                qkv = qp.tile([128, 3 * D], f32, tag="qkv")
                nc.scalar.copy(qkv[:], qkv_ps[:])
                q_ap = qkv[:, 0:D]
                k_ap = qkv[:, D:2 * D]
                q_bc = bass.AP(tensor=q_ap.tensor, offset=q_ap.offset,
                               ap=[q_ap.ap[0], [DH, H], [1, DH], [0, DH]])
                k_bc = bass.AP(tensor=k_ap.tensor, offset=k_ap.offset,
                               ap=[k_ap.ap[0], [DH, H], [0, DH], [1, DH]])
                P = bigp.tile([128, H * DH * DH], f32, tag="P")
                nc.vector.tensor_mul(
                    P[:].rearrange("p (h i j) -> p h i j", h=H, i=DH), q_bc, k_bc)
                E = bige.tile([128, H * DH * DH], f32, tag="E")
                nc.scalar.activation(E[:], P[:], AF.Exp, bias=zero_col[:], scale=SCALE)
                return {"qkv": qkv, "P": P, "E": E}

            def stage_b(st):
                S = smp.tile([128, D], f32, tag="S")
                nc.vector.tensor_reduce(
                    S[:], st["E"][:].rearrange("p (a j) -> p a j", j=DH),
                    axis=AX.X, op=ALU.add)
                v_ap = st["qkv"][:, 2 * D:3 * D]
                v_bc = bass.AP(tensor=v_ap.tensor, offset=v_ap.offset,
                               ap=[v_ap.ap[0], [DH, H], [0, DH], [1, DH]])
                # EV into P's slot (dead after exp); GPSIMD is otherwise idle.
                nc.gpsimd.tensor_mul(
                    st["P"][:].rearrange("p (h i j) -> p h i j", h=H, i=DH),
                    st["E"][:].rearrange("p (h i j) -> p h i j", h=H, i=DH), v_bc)
                lnS = smp.tile([128, D], f32, tag="lnS")
                nc.scalar.activation(lnS[:], S[:], AF.Ln, bias=zero_col[:])
                rS = smp.tile([128, D], f32, tag="rS")
                nc.scalar.activation(rS[:], lnS[:], AF.Exp, bias=zero_col[:], scale=-1.0)
                st["rS"] = rS

            def stage_c(st, ofm, loc):
                O = smp.tile([128, D], f32, tag="O")
                nc.vector.tensor_reduce(
                    O[:], st["P"][:].rearrange("p (a j) -> p a j", j=DH),
                    axis=AX.X, op=ALU.add)
                On = smp.tile([128, D], f32, tag="On")
                nc.vector.tensor_mul(On[:], O[:], st["rS"][:])
                ot_ps = psC.tile([128, D], f32, tag="psc")
                nc.tensor.transpose(ot_ps[:], On[:], ident[:])
                nc.scalar.copy(ofm[:, loc * 128:(loc + 1) * 128], ot_ps[:])

            # ---------------- per-macro-chunk network body ----------------
            def stream(h0_ap, b0):
                h_ap = h0_ap
                for blk in range(NB):
                    y = layernorm(h_ap, MCB)
                    ofm = ofp.tile([D, MCB], f32, tag="ofm")
                    # Cross-pair stagger: pair-2's q*k products (DVE) fill the
                    # window where pair-1's O-reduces wait on the serialized
                    # GPSIMD E*v multiplies.
                    sts = {}
                    sts[0] = stage_a(y, 0, blk)
                    sts[1] = stage_a(y, 1, blk)
                    stage_b(sts[0])
                    stage_b(sts[1])
                    sts[2] = stage_a(y, 2, blk)
                    sts[3] = stage_a(y, 3, blk)
                    stage_c(sts[0], ofm, 0)
                    stage_c(sts[1], ofm, 1)
                    stage_b(sts[2])
                    stage_b(sts[3])
                    stage_c(sts[2], ofm, 2)
                    stage_c(sts[3], ofm, 3)
                    wo_ps = psA.tile([128, MCB], f32, tag="psa")
                    nc.tensor.matmul(wo_ps[:], wo_sb[:, blk, :], ofm[:],
                                     start=True, stop=skip_bo)
                    if not skip_bo:
                        nc.tensor.matmul(wo_ps[:], bo_sb[:, blk, :], ones512[:],
                                         start=False, stop=True)
                    h2 = hp.tile([D, MCB], f32, tag="h")
                    nc.vector.tensor_add(h2[:], h_ap, wo_ps[:])
                    h_ap = h2[:]
                    y2 = layernorm(h_ap, MCB)
                    ffs = ffp.tile([128, 4, MCB], f32, tag="ffs")
                    w2_ps = psA.tile([128, MCB], f32, tag="psa")
                    for c in range(4):
                        ff_ps = psB.tile([128, MCB], f32, tag="psb")
                        nc.tensor.matmul(ff_ps[:], w1_sb[:, blk, c * 128:(c + 1) * 128],
                                         y2[:], start=True, stop=True)
                        if skip_b1:
                            nc.vector.tensor_relu(ffs[:, c, :], ff_ps[:])
                        else:
                            nc.scalar.activation(ffs[:, c, :], ff_ps[:], AF.Relu,
                                                 bias=b1_sb[:, blk, c, :])
                        nc.tensor.matmul(w2_ps[:], w2_sb[:, blk, c, :], ffs[:, c, :],
                                         start=(c == 0), stop=(c == 3 and skip_b2))
                    if not skip_b2:
                        nc.tensor.matmul(w2_ps[:], b2_sb[:, blk, :], ones512[:],
                                         start=False, stop=True)
                    h3 = hp.tile([D, MCB], f32, tag="h")
                    nc.vector.tensor_add(h3[:], h_ap, w2_ps[:])
                    h_ap = h3[:]
                # heads
                val_ps = psC.tile([1, MCB], f32, tag="psc")
                nc.tensor.matmul(val_ps[:], valw_sb[:], h_ap, start=True, stop=True)
                val_sb = lns.tile([1, MCB], f32, tag="val")
                nc.scalar.activation(val_sb[:], val_ps[:], AF.Identity, bias=valb_sb[:])
                nc.sync.dma_start(value_d[b0:b0 + MCB, :], val_sb[:])
                with tc.high_priority(offset=-150):
                    for s in range(4):
                        for n0, nw in V_CHUNKS:
                            pol_ps = psD.tile([128, 512], f32, tag="psd")
                            nc.tensor.matmul(pol_ps[:, :nw],
                                             h_ap[:, s * 128:(s + 1) * 128],
                                             polw_eff[:, n0:n0 + nw], start=True,
                                             stop=skip_polb)
                            if not skip_polb:
                                nc.tensor.matmul(pol_ps[:, :nw], ones_row[:],
                                                 polb_eff[:, n0:n0 + nw], start=False,
                                                 stop=True)
                            pol_sb = ofp.tile([128, 512], f32, tag="pol")
                            nc.scalar.copy(pol_sb[:, :nw], pol_ps[:, :nw])
                            nc.sync.dma_start(
                                policy_d[b0 + s * 128: b0 + (s + 1) * 128, n0:n0 + nw],
                                pol_sb[:, :nw])

            # ---------------- main loop ----------------
            for mc in range(n_mc):
                b0 = mc * MCB
                # Emit the load/transpose/encode front-end "earlier" so it fills
                # the previous macro-chunk's FF/LN engine bubbles.
                with tc.high_priority(offset=(250 if mc > 0 else None)):
                    x_sb = xp.tile([128, 4, DIN], f32, tag="x")
                    for s in range(4):
                        nc.sync.dma_start(x_sb[:, s, :],
                                          x_d[b0 + s * 128: b0 + (s + 1) * 128, :])
                    xt_sb = xtp.tile([128, 4, MCB], f32, tag="xt")
                    for c, (c0, cw) in enumerate(DIN_CHUNKS):
                        xt_ps = psA.tile([128, 512], f32, tag="psa")
                        for s in range(4):
                            nc.tensor.transpose(xt_ps[:cw, s * 128:(s + 1) * 128],
                                                x_sb[:, s, c0:c0 + cw], ident[:])
                        nc.scalar.copy(xt_sb[:cw, c, :], xt_ps[:cw, :])
                    h_ps = psA.tile([128, 512], f32, tag="psa")
                    for c, (c0, cw) in enumerate(DIN_CHUNKS):
                        nc.tensor.matmul(h_ps[:], encw_sb[:cw, c, :], xt_sb[:cw, c, :],
                                         start=(c == 0), stop=(c == 3))
                    henc = hp.tile([D, MCB], f32, tag="henc")
                    nc.scalar.activation(henc[:], h_ps[:], AF.Relu, bias=encb_sb[:])

                stream(henc[:], b0)

            _main_pools.close()

    # Steer the ACT table-set chooser to the one set that contains every
    # function this kernel uses (Exp, Ln, Relu, Copy, Identity).  Without
    # this, Exp picks exp_and_others and each Ln forces a ~1.3us table
    # reload (193 reloads = ~250us wasted on the scalar engine).  Pruning
    # our functions from every other set leaves the chooser a unique
    # choice; dict order (= act_func_set_id mapping) is unchanged.
    from concourse import hw_specs
    import concourse.bacc as bacc_mod
    orig_fn = hw_specs.get_activation_tables
    keep = "natural_log_exp_and_others"
    mine = {AF.Exp, AF.Ln, AF.Relu, AF.Copy, AF.Identity, AF.Square,
            AF.MemsetZero, AF.Abs, AF.Sign, AF.Is_finite, AF.Prelu}

    def patched(arch):
        t = {k: set(v) for k, v in orig_fn(arch).items()}
        assert mine & t[keep] >= {AF.Exp, AF.Ln, AF.Relu, AF.Copy, AF.Identity}
        for k in t:
            if k != keep:
                t[k] = t[k] - mine
        return t

    bacc_mod.get_activation_tables = patched
    hw_specs_orig = hw_specs.get_activation_tables
    hw_specs.get_activation_tables = patched
    try:
        nc.compile()
    finally:
        bacc_mod.get_activation_tables = orig_fn
        hw_specs.get_activation_tables = hw_specs_orig
    return nc


def _prep_host(inputs):
    g = lambda k: np.asarray(inputs[k], dtype=np.float32)
    x = g("x")
    Wq, bq = g("Wq"), g("bq")
    Wk, bk = g("Wk"), g("bk")
    Wv, bv = g("Wv"), g("bv")
    Wo, bo = g("Wo"), g("bo")
    ln1_g, ln1_b = g("ln1_g"), g("ln1_b")
    ln2_g, ln2_b = g("ln2_g"), g("ln2_b")
    W1, b1 = g("W1"), g("b1")
    W2, b2 = g("W2"), g("b2")

    wqkv = np.zeros((NB, D, 3 * D), np.float32)
    bqkv = np.zeros((NB, 1, 3 * D), np.float32)
    w1e = np.zeros((NB, D, FF), np.float32)
    b1e = np.zeros((NB, FF, 1), np.float32)
    for i in range(NB):
        cat_raw = np.concatenate(
            [np.transpose(Wq[i], (1, 0, 2)).reshape(D, D),
             np.transpose(Wk[i], (1, 0, 2)).reshape(D, D),
             np.transpose(Wv[i], (1, 0, 2)).reshape(D, D)], axis=1)   # [D, 384]
        wqkv[i] = ln1_g[i][:, None] * cat_raw
        bqkv[i, 0] = (np.concatenate([bq[i].ravel(), bk[i].ravel(), bv[i].ravel()])
                      + ln1_b[i] @ cat_raw)
        w1e[i] = ln2_g[i][:, None] * W1[i]
        b1e[i, :, 0] = ln2_b[i] @ W1[i] + b1[i]

    arrs = {
        "x": x,
        "encw": g("enc_w"),
        "encb": g("enc_b").reshape(D, 1),
        "wqkv": wqkv,
        "bqkv": bqkv,
        "wo": Wo,
        "bo": bo.reshape(NB, 1, D),
        "w1": w1e,
        "b1": b1e,
        "w2": W2,
        "b2": b2.reshape(NB, 1, D),
        "valw": g("val_w"),
        "valb": g("val_b").reshape(1, 1),
        "polwT": np.ascontiguousarray(g("pol_w").T),
        "polb": g("pol_b").reshape(POL, 1),
        "wmT": np.ascontiguousarray(g("word_matrix").T),
    }
    arrs = {k: np.ascontiguousarray(v, dtype=np.float32) for k, v in arrs.items()}
    skips = (bool(np.all(bqkv == 0.0)), bool(np.all(bo == 0.0)),
             bool(np.all(b2 == 0.0)), bool(np.all(arrs["polb"] == 0.0)),
             bool(np.all(b1e == 0.0)))
    return arrs, skips


def kernel(**inputs):
    from concourse.bass_utils import run_bass_kernel_spmd

    arrs, skips = _prep_host(inputs)
    key = (BC,) + skips
    if key not in _PROGRAM_CACHE:
        _PROGRAM_CACHE[key] = _build_program(BC, *skips)
    nc = _PROGRAM_CACHE[key]

    weights = {k: v for k, v in arrs.items() if k != "x"}
    in_maps = []
    for c in range(NCORES):
        m = dict(weights)
        m["x"] = np.ascontiguousarray(arrs["x"][c * BC:(c + 1) * BC])
        in_maps.append(m)

    res = run_bass_kernel_spmd(nc, in_maps, core_ids=list(range(NCORES)),
                               trace=bool(int(os.environ.get("KERNEL_TRACE", "0"))))
    policy = np.concatenate([r["policy"] for r in res.results], axis=0)
    value = np.concatenate([r["value"] for r in res.results], axis=0)
    if res.exec_time_ns is not None:
        print(f"HW exec time: {res.exec_time_ns} ns")
    kernel.last_results = res
    return policy, value


kernel.last_results = None
